# revision 7
# baseline (speedup 1.0000x reference)
"""Trainium2 Bass kernel for int8-valued Conv2d(128->256, 3x3, pad 1) + BN-add +
shift requant + clip + uint8 cast, over x[32,128,56,56].

Strategy: data-parallel over batch across 8 NeuronCores (4 images/core), with a
1D Winograd F(2,3) transform along H. Per output row-pair the three vertical
taps collapse into 4 transformed planes (m0..m3), each needing only the 3
horizontal-tap matmuls: 12 matmuls per 16-row chunk instead of 18 -> PE work
drops to 2/3 of the direct conv.

Transformed weights are fp16 (half-integers up to 190.5 are exact; bf16 is
not enough) pre-scaled by 2^-10 so the BN right-shift happens inside the
matmul for free. Transformed inputs (int combos up to +-256) are exact in
fp16. All PSUM sums stay on the 2^-11 grid below 2^13, so fp32 accumulation
is bit-exact (validated against the reference in numpy).

Inverse transform + requant per chunk (planes m0,m1,m2,-m3 in PSUM):
  ACT : mm1 = m1, mm2 = m2 (PSUM->SBUF copies)
  DVE : e = (m0 + tb) + mm1 ; o = (-m3 + tb) + mm1     (scalar_tensor_tensor)
  GpS : ye = e + mm2 ; yo = o - mm2                    (even / odd rows)
  DVE : clamp to [lo,hi] -> uint8 (interleaved rows), then one DMA per chunk
where tb = t*2^-10 - (0.5 - 2^-11). The hardware fp32->uint8 convert rounds
to nearest (probed); the -0.5+2^-11 bias makes round(x) == floor of the true
shifted value, and ties cannot occur (results sit on the 2^-10 grid).
"""

import numpy as np
import ml_dtypes
from contextlib import ExitStack

import concourse.bass as bass  # noqa: F401  (registers engine types)
import concourse.mybir as mybir
import concourse.tile as tile
from concourse import bacc
from concourse.bass_utils import run_bass_kernel_spmd

# Problem constants (hardcoded per contract)
N_CORES = 8
B = 32
B_LOC = B // N_CORES          # 4 images per core
P = 128                       # Cin = partition dim
H = W = 56
COUT = 256
G = COUT // P                 # 2 Cout halves
PAIRS = H // 2                # 28 output row-pairs per image
WT = 58                       # transformed row width (56 + 2 zero pad cols)
PLANE = PAIRS * WT            # 1624 elems per transformed plane
IMG = H * W                   # compact image elems per partition
# chunks of output row-pairs per (img, g): 8+8+8+4 pairs = 56 rows
CHUNKS = [(0, 8), (8, 8), (16, 8), (24, 4)]

_cache = {}


def _build(shift: int):
    """Build + compile the per-core Bass program. Same NEFF on all 8 cores."""
    nc = bacc.Bacc("TRN2", target_bir_lowering=False, debug=False,
                   num_devices=N_CORES)

    xs = nc.dram_tensor("xs", [B_LOC, P, H, W], mybir.dt.int8, kind="ExternalInput")
    wt = nc.dram_tensor("wt", [P, 4 * 3 * COUT], mybir.dt.float16, kind="ExternalInput")
    tb = nc.dram_tensor("tb", [P, G], mybir.dt.float32, kind="ExternalInput")
    lo = nc.dram_tensor("lo", [P, G], mybir.dt.float32, kind="ExternalInput")
    hi = nc.dram_tensor("hi", [P, G], mybir.dt.float32, kind="ExternalInput")
    ys = nc.dram_tensor("ys", [B_LOC, COUT, H, W], mybir.dt.uint8, kind="ExternalOutput")

    with tile.TileContext(nc) as tc, ExitStack() as ctx:
        wpool = ctx.enter_context(tc.tile_pool(name="wpool", bufs=1))
        cpool = ctx.enter_context(tc.tile_pool(name="cpool", bufs=1))
        xbpool = ctx.enter_context(tc.tile_pool(name="xbpool", bufs=1))
        xtpool = ctx.enter_context(tc.tile_pool(name="xtpool", bufs=2))
        pspool = ctx.enter_context(tc.tile_pool(name="pspool", bufs=2, space="PSUM"))
        mpool = ctx.enter_context(tc.tile_pool(name="mpool", bufs=4))
        eopool = ctx.enter_context(tc.tile_pool(name="eopool", bufs=4))
        yepool = ctx.enter_context(tc.tile_pool(name="yepool", bufs=4))
        opool = ctx.enter_context(tc.tile_pool(name="opool", bufs=4))
        zpool = ctx.enter_context(tc.tile_pool(name="zpool", bufs=1))

        wt_s = wpool.tile([P, 4 * 3 * COUT], mybir.dt.float16)
        tb_s = cpool.tile([P, G], mybir.dt.float32)
        lo_s = cpool.tile([P, G], mybir.dt.float32)
        hi_s = cpool.tile([P, G], mybir.dt.float32)
        # all 4 images, compact rows, int8 (DMA lands here directly)
        xbig = xbpool.tile([P, B_LOC * IMG], mybir.dt.int8)
        zrow = zpool.tile([P, W], mybir.dt.int8)

        # HAM pre-warm: the PE is idle ~4us at start while input DMAs run.
        # A stream of zero matmuls gets the clock gate to K=8/8 (2.4 GHz)
        # before the first real matmul; no data deps.
        zs = zpool.tile([P, 384], mybir.dt.float16)
        nc.vector.memset(zs[:], 0.0)
        nc.vector.memset(zrow[:], 0)
        wps = pspool.tile([P, 464], mybir.dt.float32, name="ps0")
        for _ in range(12):
            nc.tensor.matmul(wps[:, :384], lhsT=zs[:, :P], rhs=zs[:],
                             start=True, stop=True)

        # ---- input loads (int8, compact) + H-transform into fp16 planes ----
        def load_rows(img, r0, nrows):
            nc.sync.dma_start(
                xbig[:, img * IMG + r0 * W: img * IMG + (r0 + nrows) * W],
                xs.ap()[img, :, r0:r0 + nrows, :].rearrange("c h w -> c (h w)"))

        xts = []   # per-image transformed tiles

        def make_xt(img):
            # layout: [1 guard col][4 planes of 28*58][1 guard col]
            xt = xtpool.tile([P, 4 * PLANE + 2], mybir.dt.float16, name="xt")
            v = xt[:, 1:1 + 4 * PLANE].rearrange("p (n w) -> p n w", w=WT)
            nc.vector.memset(v[:, :, 0:1], 0.0)        # pad col 0, all planes
            nc.vector.memset(v[:, :, WT - 1:WT], 0.0)  # pad col 57
            nc.vector.memset(xt[:, 0:1], 0.0)          # leading dw guard
            nc.vector.memset(xt[:, 1 + 4 * PLANE:], 0.0)  # trailing dw guard
            return xt

        # plane defs: (k, row_offset_in0, row_offset_in1, op) with rows as
        # input-row index of pair r: in = x[off + 2r]; op in {subtract, add}
        # m0: x[2r-1] - x[2r+1] ; m1: x[2r] + x[2r+1]
        # m2: x[2r+1] - x[2r]   ; m3n = -(x[2r] - x[2r+3... x[2r+2]]) built
        # directly as x[2r+2] - x[2r]  (weights for plane 3 are NOT negated
        # on the host; instead m3n = -m3 is produced by swapping operands).
        sub = mybir.AluOpType.subtract
        add = mybir.AluOpType.add

        def transform(img, xt, p0, np_):
            """Emit transform for pairs [p0, p0+np_) of all 4 planes."""
            # [P, 28, 2, 56]: input row 2q+d = xv2[:, q, d, :]
            xv2 = xbig[:, img * IMG:(img + 1) * IMG].rearrange(
                "p (r two w) -> p r two w", two=2, w=W)

            def rv(row, n):   # input rows row, row+2, ... (n of them)
                q, d = divmod(row, 2)
                return xv2[:, q:q + n, d:d + 1, :].rearrange(
                    "p r one w -> p (r one) w")

            def ov(k, q0, n):
                s = xt[:, 1 + k * PLANE + q0 * WT:
                       1 + k * PLANE + (q0 + n) * WT]
                return s.rearrange("p (r w) -> p r w", w=WT)[:, :, 1:1 + W]

            zv = zrow[:].rearrange("p (r w) -> p r w", w=W)
            for k, (off0, off1, op) in enumerate([(-1, 1, sub), (0, 1, add),
                                                  (1, 0, sub), (2, 0, sub)]):
                q0, n = p0, np_
                if k == 0 and q0 == 0:      # pair 0 uses x[-1] = 0
                    nc.vector.tensor_tensor(ov(0, 0, 1), zv, rv(1, 1), sub)
                    q0, n = 1, n - 1
                if k == 3 and q0 + n == PAIRS:   # pair 27 uses x[56] = 0
                    nc.vector.tensor_tensor(
                        ov(3, PAIRS - 1, 1), zv, rv(2 * (PAIRS - 1), 1), sub)
                    n = n - 1
                if n <= 0:
                    continue
                nc.vector.tensor_tensor(
                    ov(k, q0, n), rv(off0 + 2 * q0, n),
                    rv(off1 + 2 * q0, n), op)

        # Load/transform schedule: quarters for img0/1 (critical path),
        # halves for img2/3. Transform halves after covering loads.
        QR = H // 4
        load_rows(0, 0, QR)
        nc.sync.dma_start(wt_s[:, :384], wt.ap()[:, :384])            # g0 k0
        nc.sync.dma_start(wt_s[:, 384:1536], wt.ap()[:, 384:1536])    # g0 k1-3
        load_rows(0, QR, QR)
        nc.sync.dma_start(tb_s[:], tb.ap())
        nc.sync.dma_start(lo_s[:], lo.ap())
        nc.sync.dma_start(hi_s[:], hi.ap())
        xt0 = make_xt(0)
        transform(0, xt0, 0, 13)
        load_rows(0, 2 * QR, QR)
        load_rows(0, 3 * QR, QR)
        transform(0, xt0, 13, 15)
        xts.append(xt0)
        load_rows(1, 0, QR)
        load_rows(1, QR, QR)
        nc.sync.dma_start(wt_s[:, 1536:], wt.ap()[:, 1536:])          # g1
        xt1 = make_xt(1)
        transform(1, xt1, 0, 13)
        load_rows(1, 2 * QR, QR)
        load_rows(1, 3 * QR, QR)
        transform(1, xt1, 13, 15)
        xts.append(xt1)
        for img in range(2, B_LOC):
            load_rows(img, 0, H // 2)
            load_rows(img, H // 2, H // 2)
            xti = make_xt(img)
            transform(img, xti, 0, 13)
            transform(img, xti, 13, 15)
            xts.append(xti)

        # ---- main loop: 12 matmuls + inverse transform per 16-row chunk ----
        for img in range(B_LOC):
            xt = xts[img]
            for g in range(G):
                for (pr0, npr) in CHUNKS:
                    ncols = npr * WT
                    ps = []
                    for k in range(4):
                        pk = pspool.tile([P, 464], mybir.dt.float32,
                                         name=f"ps{k}")
                        for dw in range(3):
                            nc.tensor.matmul(
                                pk[:, :ncols],
                                lhsT=wt_s[:, ((g * 4 + k) * 3 + dw) * P:
                                          ((g * 4 + k) * 3 + dw + 1) * P],
                                rhs=xt[:, k * PLANE + pr0 * WT + dw:
                                       k * PLANE + pr0 * WT + dw + ncols],
                                start=(dw == 0),
                                stop=(dw == 2),
                            )
                        ps.append(pk)
                    mm1 = mpool.tile([P, 464], mybir.dt.float32, name="mm1")
                    mm2 = mpool.tile([P, 464], mybir.dt.float32, name="mm2")
                    nc.scalar.copy(mm1[:, :ncols], ps[1][:, :ncols])
                    nc.scalar.copy(mm2[:, :ncols], ps[2][:, :ncols])
                    e = eopool.tile([P, 464], mybir.dt.float32, name="e")
                    o = eopool.tile([P, 464], mybir.dt.float32, name="o")
                    nc.vector.scalar_tensor_tensor(
                        e[:, :ncols], ps[0][:, :ncols], tb_s[:, g:g + 1],
                        mm1[:, :ncols], add, add)
                    nc.vector.scalar_tensor_tensor(
                        o[:, :ncols], ps[3][:, :ncols], tb_s[:, g:g + 1],
                        mm1[:, :ncols], add, add)
                    ye = yepool.tile([P, 464], mybir.dt.float32, name="ye")
                    yo = yepool.tile([P, 464], mybir.dt.float32, name="yo")
                    nc.gpsimd.tensor_tensor(ye[:, :ncols], e[:, :ncols],
                                            mm2[:, :ncols], add)
                    nc.gpsimd.tensor_tensor(yo[:, :ncols], o[:, :ncols],
                                            mm2[:, :ncols], sub)
                    # clamp + compact pad cols + interleave even/odd rows
                    ot = opool.tile([P, 16 * W], mybir.dt.uint8, name="ot")
                    otv = ot[:, :npr * 2 * W].rearrange(
                        "p (r two w) -> p r two w", two=2, w=W)
                    yev = ye[:, :ncols].rearrange(
                        "p (r w) -> p r w", w=WT)[:, :, 1:1 + W]
                    yov = yo[:, :ncols].rearrange(
                        "p (r w) -> p r w", w=WT)[:, :, 1:1 + W]
                    nc.vector.tensor_scalar(
                        otv[:, :, 0, :], yev, lo_s[:, g:g + 1],
                        hi_s[:, g:g + 1], mybir.AluOpType.max,
                        mybir.AluOpType.min)
                    nc.vector.tensor_scalar(
                        otv[:, :, 1, :], yov, lo_s[:, g:g + 1],
                        hi_s[:, g:g + 1], mybir.AluOpType.max,
                        mybir.AluOpType.min)
                    nc.sync.dma_start(
                        ys.ap()[img, g * P:(g + 1) * P,
                                2 * pr0:2 * (pr0 + npr), :]
                        .rearrange("c h w -> c (h w)"),
                        ot[:, :npr * 2 * W])

    nc.compile()
    return nc


def _pack_inputs(x, weight, t, n, act_min, act_max):
    x = np.asarray(x)
    weight = np.asarray(weight)
    t = np.asarray(t).reshape(COUT)
    n = np.asarray(n).reshape(COUT)
    act_min = np.asarray(act_min).reshape(COUT)
    act_max = np.asarray(act_max).reshape(COUT)

    assert x.shape == (B, P, H, W) and weight.shape == (COUT, P, 3, 3)
    nval = int(n[0])
    assert np.all(n == nval) and nval <= 0, "non-uniform/positive BN shift unsupported"
    shift = -nval
    assert np.all(act_min >= 0) and np.all(act_max <= 255), \
        "act range must fit uint8 (pure_positive path)"
    assert x.min() >= -128 and x.max() <= 127
    assert np.abs(weight).max(initial=0) <= 128

    scale = np.float64(2.0) ** -shift
    # H-transform of weights: planes (g0, g1, g2, g3) stacked, then dw, g, co
    W0 = weight[:, :, 0, :].astype(np.float64)
    W1 = weight[:, :, 1, :].astype(np.float64)
    W2 = weight[:, :, 2, :].astype(np.float64)
    planes = np.stack([W0, (W0 + W1 + W2) / 2, (W0 - W1 + W2) / 2, W2],
                      axis=0) * scale                      # [4, COUT, P, 3]
    # lhsT layout: wt[ci, ((g*4 + k)*3 + dw)*P + co]
    pr = planes.reshape(4, G, P, P, 3)                     # [k, g, co, ci, dw]
    pr = pr.transpose(3, 1, 0, 4, 2)                       # [ci, g, k, dw, co]
    wt_np = np.ascontiguousarray(
        pr.reshape(P, 4 * 3 * COUT)).astype(np.float16)
    assert np.array_equal(
        wt_np.astype(np.float64).reshape(P, G, 4, 3, P).transpose(2, 1, 4, 0, 3),
        planes.reshape(4, G, P, P, 3)), "weight transform not fp16-exact"

    c_off = 0.5 - 2.0 ** -11   # round-to-nearest -> floor (no ties possible)
    tb_np = np.ascontiguousarray(
        (t.astype(np.float64) * scale - c_off).reshape(G, P).T
    ).astype(np.float32)
    lo_np = np.ascontiguousarray(act_min.reshape(G, P).T).astype(np.float32)
    hi_np = np.ascontiguousarray(act_max.reshape(G, P).T).astype(np.float32)
    return x, wt_np, tb_np, lo_np, hi_np, shift


def kernel(x, weight, t, n, act_min, act_max):
    x, wt_np, tb_np, lo_np, hi_np, shift = _pack_inputs(
        x, weight, t, n, act_min, act_max)

    if shift not in _cache:
        _cache[shift] = _build(shift)
    nc = _cache[shift]

    x8 = x.astype(np.int8)  # exact: setup guarantees int8-valued data
    in_maps = []
    for c in range(N_CORES):
        in_maps.append({
            "xs": np.ascontiguousarray(x8[c * B_LOC:(c + 1) * B_LOC]),
            "wt": wt_np,
            "tb": tb_np,
            "lo": lo_np,
            "hi": hi_np,
        })
    res = run_bass_kernel_spmd(nc, in_maps, core_ids=list(range(N_CORES)))
    out = np.concatenate([res.results[c]["ys"] for c in range(N_CORES)], axis=0)
    return out


# revision 9
# speedup vs baseline: 1.2902x; 1.2902x over previous
"""Trainium2 Bass kernel for int8-valued Conv2d(128->256, 3x3, pad 1) + BN-add +
shift requant + clip + uint8 cast, over x[32,128,56,56].

Strategy: data-parallel over batch across 8 NeuronCores (4 images/core), with a
1D Winograd F(2,3) transform along H. Per output row-pair the three vertical
taps collapse into 4 transformed planes (m0..m3), each needing only the 3
horizontal-tap matmuls: 12 matmuls per 16-row chunk instead of 18 -> PE work
drops to 2/3 of the direct conv.

Transformed weights are fp16 (half-integers up to 190.5 are exact; bf16 is
not enough) pre-scaled by 2^-10 so the BN right-shift happens inside the
matmul for free. Transformed inputs (int combos up to +-256) are exact in
fp16. All PSUM sums stay on the 2^-11 grid below 2^13, so fp32 accumulation
is bit-exact (validated against the reference in numpy).

Inverse transform + requant per chunk (planes m0,m1,m2,-m3 in PSUM):
  ACT : mm1 = m1, mm2 = m2 (PSUM->SBUF copies)
  DVE : e = (m0 + tb) + mm1 ; o = (-m3 + tb) + mm1     (scalar_tensor_tensor)
  GpS : ye = e + mm2 ; yo = o - mm2                    (even / odd rows)
  DVE : clamp to [lo,hi] -> uint8 (interleaved rows), then one DMA per chunk
where tb = t*2^-10 - (0.5 - 2^-11). The hardware fp32->uint8 convert rounds
to nearest (probed); the -0.5+2^-11 bias makes round(x) == floor of the true
shifted value, and ties cannot occur (results sit on the 2^-10 grid).
"""

import numpy as np
import ml_dtypes
from contextlib import ExitStack

import concourse.bass as bass  # noqa: F401  (registers engine types)
import concourse.mybir as mybir
import concourse.tile as tile
from concourse import bacc
from concourse.bass_utils import run_bass_kernel_spmd

# Problem constants (hardcoded per contract)
N_CORES = 8
B = 32
B_LOC = B // N_CORES          # 4 images per core
P = 128                       # Cin = partition dim
H = W = 56
COUT = 256
G = COUT // P                 # 2 Cout halves
PAIRS = H // 2                # 28 output row-pairs per image
WT = 58                       # transformed row width (56 + 2 zero pad cols)
PLANE = PAIRS * WT            # 1624 elems per transformed plane
IMG = H * W                   # compact image elems per partition
# chunks of output row-pairs per (img, g): 8+8+8+4 pairs = 56 rows
CHUNKS = [(0, 8), (8, 8), (16, 8), (24, 4)]

_cache = {}


def _build(shift: int):
    """Build + compile the per-core Bass program. Same NEFF on all 8 cores."""
    nc = bacc.Bacc("TRN2", target_bir_lowering=False, debug=False,
                   num_devices=N_CORES)

    xs = nc.dram_tensor("xs", [B_LOC, P, H, W], mybir.dt.int8, kind="ExternalInput")
    wt = nc.dram_tensor("wt", [P, 4 * 3 * COUT], mybir.dt.float16, kind="ExternalInput")
    tb = nc.dram_tensor("tb", [P, G], mybir.dt.float32, kind="ExternalInput")
    lo = nc.dram_tensor("lo", [P, G], mybir.dt.float32, kind="ExternalInput")
    hi = nc.dram_tensor("hi", [P, G], mybir.dt.float32, kind="ExternalInput")
    ys = nc.dram_tensor("ys", [B_LOC, COUT, H, W], mybir.dt.uint8, kind="ExternalOutput")

    with tile.TileContext(nc) as tc, ExitStack() as ctx:
        wpool = ctx.enter_context(tc.tile_pool(name="wpool", bufs=1))
        cpool = ctx.enter_context(tc.tile_pool(name="cpool", bufs=1))
        xbpool = ctx.enter_context(tc.tile_pool(name="xbpool", bufs=1))
        xtpool = ctx.enter_context(tc.tile_pool(name="xtpool", bufs=2))
        pspool = ctx.enter_context(tc.tile_pool(name="pspool", bufs=2, space="PSUM"))
        mpool = ctx.enter_context(tc.tile_pool(name="mpool", bufs=4))
        eopool = ctx.enter_context(tc.tile_pool(name="eopool", bufs=4))
        opool = ctx.enter_context(tc.tile_pool(name="opool", bufs=4))
        zpool = ctx.enter_context(tc.tile_pool(name="zpool", bufs=1))

        wt_s = wpool.tile([P, 4 * 3 * COUT], mybir.dt.float16)
        tb_s = cpool.tile([P, G], mybir.dt.float32)
        lo_s = cpool.tile([P, G], mybir.dt.float32)
        hi_s = cpool.tile([P, G], mybir.dt.float32)
        # all 4 images, compact rows, int8 (DMA lands here directly)
        xbig = xbpool.tile([P, B_LOC * IMG], mybir.dt.int8)
        zrow = zpool.tile([P, W], mybir.dt.int8)

        # HAM pre-warm: the PE is idle ~4us at start while input DMAs run.
        # A stream of zero matmuls gets the clock gate to K=8/8 (2.4 GHz)
        # before the first real matmul; no data deps.
        zs = zpool.tile([P, 384], mybir.dt.float16)
        nc.vector.memset(zs[:], 0.0)
        nc.vector.memset(zrow[:], 0)
        wps = pspool.tile([P, 464], mybir.dt.float32, name="ps0")
        for _ in range(12):
            nc.tensor.matmul(wps[:, :384], lhsT=zs[:, :P], rhs=zs[:],
                             start=True, stop=True)

        # ---- input loads (int8, compact) + H-transform into fp16 planes ----
        def load_rows(img, r0, nrows):
            nc.sync.dma_start(
                xbig[:, img * IMG + r0 * W: img * IMG + (r0 + nrows) * W],
                xs.ap()[img, :, r0:r0 + nrows, :].rearrange("c h w -> c (h w)"))

        xts = []   # per-image transformed tiles

        def make_xt(img):
            # layout: [1 guard col][4 planes of 28*58][1 guard col]
            xt = xtpool.tile([P, 4 * PLANE + 2], mybir.dt.float16, name="xt")
            v = xt[:, 1:1 + 4 * PLANE].rearrange("p (n w) -> p n w", w=WT)
            nc.vector.memset(v[:, :, 0:1], 0.0)        # pad col 0, all planes
            nc.vector.memset(v[:, :, WT - 1:WT], 0.0)  # pad col 57
            nc.vector.memset(xt[:, 0:1], 0.0)          # leading dw guard
            nc.vector.memset(xt[:, 1 + 4 * PLANE:], 0.0)  # trailing dw guard
            return xt

        # plane defs: (k, row_offset_in0, row_offset_in1, op) with rows as
        # input-row index of pair r: in = x[off + 2r]; op in {subtract, add}
        # m0: x[2r-1] - x[2r+1] ; m1: x[2r] + x[2r+1]
        # m2: x[2r+1] - x[2r]   ; m3n = -(x[2r] - x[2r+3... x[2r+2]]) built
        # directly as x[2r+2] - x[2r]  (weights for plane 3 are NOT negated
        # on the host; instead m3n = -m3 is produced by swapping operands).
        sub = mybir.AluOpType.subtract
        add = mybir.AluOpType.add

        def transform(img, xt, p0, np_):
            """Emit transform for pairs [p0, p0+np_) of all 4 planes."""
            # [P, 28, 2, 56]: input row 2q+d = xv2[:, q, d, :]
            xv2 = xbig[:, img * IMG:(img + 1) * IMG].rearrange(
                "p (r two w) -> p r two w", two=2, w=W)

            def rv(row, n):   # input rows row, row+2, ... (n of them)
                q, d = divmod(row, 2)
                return xv2[:, q:q + n, d:d + 1, :].rearrange(
                    "p r one w -> p (r one) w")

            def ov(k, q0, n):
                s = xt[:, 1 + k * PLANE + q0 * WT:
                       1 + k * PLANE + (q0 + n) * WT]
                return s.rearrange("p (r w) -> p r w", w=WT)[:, :, 1:1 + W]

            zv = zrow[:].rearrange("p (r w) -> p r w", w=W)
            for k, (off0, off1, op) in enumerate([(-1, 1, sub), (0, 1, add),
                                                  (1, 0, sub), (2, 0, sub)]):
                q0, n = p0, np_
                if k == 0 and q0 == 0:      # pair 0 uses x[-1] = 0
                    nc.vector.tensor_tensor(ov(0, 0, 1), zv, rv(1, 1), sub)
                    q0, n = 1, n - 1
                if k == 3 and q0 + n == PAIRS:   # pair 27 uses x[56] = 0
                    nc.vector.tensor_tensor(
                        ov(3, PAIRS - 1, 1), zv, rv(2 * (PAIRS - 1), 1), sub)
                    n = n - 1
                if n <= 0:
                    continue
                nc.vector.tensor_tensor(
                    ov(k, q0, n), rv(off0 + 2 * q0, n),
                    rv(off1 + 2 * q0, n), op)

        # Load/transform schedule: quarters for img0/1 (critical path),
        # halves for img2/3. Transform halves after covering loads.
        QR = H // 4
        load_rows(0, 0, QR)
        nc.sync.dma_start(wt_s[:, :384], wt.ap()[:, :384])            # g0 k0
        nc.sync.dma_start(wt_s[:, 384:1536], wt.ap()[:, 384:1536])    # g0 k1-3
        load_rows(0, QR, QR)
        nc.sync.dma_start(tb_s[:], tb.ap())
        nc.sync.dma_start(lo_s[:], lo.ap())
        nc.sync.dma_start(hi_s[:], hi.ap())
        xt0 = make_xt(0)
        transform(0, xt0, 0, 13)
        load_rows(0, 2 * QR, QR)
        load_rows(0, 3 * QR, QR)
        transform(0, xt0, 13, 15)
        xts.append(xt0)
        load_rows(1, 0, QR)
        load_rows(1, QR, QR)
        nc.sync.dma_start(wt_s[:, 1536:], wt.ap()[:, 1536:])          # g1
        xt1 = make_xt(1)
        transform(1, xt1, 0, 13)
        load_rows(1, 2 * QR, QR)
        load_rows(1, 3 * QR, QR)
        transform(1, xt1, 13, 15)
        xts.append(xt1)
        for img in range(2, B_LOC):
            load_rows(img, 0, H // 2)
            load_rows(img, H // 2, H // 2)
            xti = make_xt(img)
            transform(img, xti, 0, 13)
            transform(img, xti, 13, 15)
            xts.append(xti)

        # ---- main loop: 12 matmuls + inverse transform per 16-row chunk ----
        for img in range(B_LOC):
            xt = xts[img]
            for g in range(G):
                for (pr0, npr) in CHUNKS:
                    ncols = npr * WT
                    ps = []
                    for k in range(4):
                        pk = pspool.tile([P, 464], mybir.dt.float32,
                                         name=f"ps{k}")
                        for dw in range(3):
                            nc.tensor.matmul(
                                pk[:, :ncols],
                                lhsT=wt_s[:, ((g * 4 + k) * 3 + dw) * P:
                                          ((g * 4 + k) * 3 + dw + 1) * P],
                                rhs=xt[:, k * PLANE + pr0 * WT + dw:
                                       k * PLANE + pr0 * WT + dw + ncols],
                                start=(dw == 0),
                                stop=(dw == 2),
                            )
                        ps.append(pk)
                    mm1 = mpool.tile([P, 464], mybir.dt.float32, name="mm1")
                    mm2 = mpool.tile([P, 464], mybir.dt.float32, name="mm2")
                    nc.scalar.copy(mm1[:, :ncols], ps[1][:, :ncols])
                    nc.scalar.copy(mm2[:, :ncols], ps[2][:, :ncols])
                    # s_e = m1 + m2 ; s_o = m1 - m2 (fp32, contiguous, GpSimd)
                    s_e = eopool.tile([P, 464], mybir.dt.float32, name="s_e")
                    s_o = eopool.tile([P, 464], mybir.dt.float32, name="s_o")
                    nc.gpsimd.tensor_tensor(s_e[:, :ncols], mm1[:, :ncols],
                                            mm2[:, :ncols], add)
                    nc.gpsimd.tensor_tensor(s_o[:, :ncols], mm1[:, :ncols],
                                            mm2[:, :ncols], sub)
                    # final: (m0 + tb) + s_e -> uint8 (saturating round; the
                    # act range [0,255] is exactly the uint8 range), writing
                    # even/odd rows interleaved + compacting the 2 pad cols.
                    ot = opool.tile([P, 16 * W], mybir.dt.uint8, name="ot")
                    otv = ot[:, :npr * 2 * W].rearrange(
                        "p (r two w) -> p r two w", two=2, w=W)

                    def win(tile56):
                        return tile56[:, :ncols].rearrange(
                            "p (r w) -> p r w", w=WT)[:, :, 1:1 + W]

                    nc.vector.scalar_tensor_tensor(
                        otv[:, :, 0, :], win(ps[0]), tb_s[:, g:g + 1],
                        win(s_e), add, add)
                    nc.vector.scalar_tensor_tensor(
                        otv[:, :, 1, :], win(ps[3]), tb_s[:, g:g + 1],
                        win(s_o), add, add)
                    nc.sync.dma_start(
                        ys.ap()[img, g * P:(g + 1) * P,
                                2 * pr0:2 * (pr0 + npr), :]
                        .rearrange("c h w -> c (h w)"),
                        ot[:, :npr * 2 * W])

    nc.compile()
    return nc


def _pack_inputs(x, weight, t, n, act_min, act_max):
    x = np.asarray(x)
    weight = np.asarray(weight)
    t = np.asarray(t).reshape(COUT)
    n = np.asarray(n).reshape(COUT)
    act_min = np.asarray(act_min).reshape(COUT)
    act_max = np.asarray(act_max).reshape(COUT)

    assert x.shape == (B, P, H, W) and weight.shape == (COUT, P, 3, 3)
    nval = int(n[0])
    assert np.all(n == nval) and nval <= 0, "non-uniform/positive BN shift unsupported"
    shift = -nval
    assert np.all(act_min == 0) and np.all(act_max == 255), \
        "saturating-uint8 requant path requires act range [0, 255]"
    assert x.min() >= -128 and x.max() <= 127
    assert np.abs(weight).max(initial=0) <= 128

    scale = np.float64(2.0) ** -shift
    # H-transform of weights: planes (g0, g1, g2, g3) stacked, then dw, g, co
    W0 = weight[:, :, 0, :].astype(np.float64)
    W1 = weight[:, :, 1, :].astype(np.float64)
    W2 = weight[:, :, 2, :].astype(np.float64)
    planes = np.stack([W0, (W0 + W1 + W2) / 2, (W0 - W1 + W2) / 2, W2],
                      axis=0) * scale                      # [4, COUT, P, 3]
    # lhsT layout: wt[ci, ((g*4 + k)*3 + dw)*P + co]
    pr = planes.reshape(4, G, P, P, 3)                     # [k, g, co, ci, dw]
    pr = pr.transpose(3, 1, 0, 4, 2)                       # [ci, g, k, dw, co]
    wt_np = np.ascontiguousarray(
        pr.reshape(P, 4 * 3 * COUT)).astype(np.float16)
    assert np.array_equal(
        wt_np.astype(np.float64).reshape(P, G, 4, 3, P).transpose(2, 1, 4, 0, 3),
        planes.reshape(4, G, P, P, 3)), "weight transform not fp16-exact"

    c_off = 0.5 - 2.0 ** -11   # round-to-nearest -> floor (no ties possible)
    tb_np = np.ascontiguousarray(
        (t.astype(np.float64) * scale - c_off).reshape(G, P).T
    ).astype(np.float32)
    lo_np = np.ascontiguousarray(act_min.reshape(G, P).T).astype(np.float32)
    hi_np = np.ascontiguousarray(act_max.reshape(G, P).T).astype(np.float32)
    return x, wt_np, tb_np, lo_np, hi_np, shift


def kernel(x, weight, t, n, act_min, act_max):
    x, wt_np, tb_np, lo_np, hi_np, shift = _pack_inputs(
        x, weight, t, n, act_min, act_max)

    if shift not in _cache:
        _cache[shift] = _build(shift)
    nc = _cache[shift]

    x8 = x.astype(np.int8)  # exact: setup guarantees int8-valued data
    in_maps = []
    for c in range(N_CORES):
        in_maps.append({
            "xs": np.ascontiguousarray(x8[c * B_LOC:(c + 1) * B_LOC]),
            "wt": wt_np,
            "tb": tb_np,
            "lo": lo_np,
            "hi": hi_np,
        })
    res = run_bass_kernel_spmd(nc, in_maps, core_ids=list(range(N_CORES)))
    out = np.concatenate([res.results[c]["ys"] for c in range(N_CORES)], axis=0)
    return out
